# revision 14
# baseline (speedup 1.0000x reference)
"""Binarized-weight 3x3 VALID conv via 1D Winograd F(2,3) along W, on 8
NeuronCores (data-parallel over batch, 4 images/core).

Per output row r and 2-col tile i (27 tiles = 54 cols):
  t0 = d[2i]-d[2i+2], t1 = d[2i+1]+d[2i+2], t2 = d[2i+2]-d[2i+1],
  t3 = d[2i+1]-d[2i+3]           (per channel, per input row; on DVE, bf16)
  P_j = sum_{c,kh} u_j[o,c,kh] * t_j[c, r+kh, i]   (PE matmuls, PSUM f32)
  out[2i] = P0+P1+P2,  out[2i+1] = P1-P2-P3        (DVE combine, f32)

u = G @ wb per kw-triple: values in {+-0.5, +-1, +-1.5} -- EXACT in
fp8-e4m3, so the stationary weights are fp8 (half-size LDWEIGHTS hides
behind the matmuls) while the moving t-planes stay bf16.

Output rows are processed in 18-row groups (486-col matmuls, one PSUM
bank per plane, 2 PSUM buffers) to halve per-instruction overhead on
every engine. x rows are split 0..37 / 36..55 so each group's 20-row
input window lives in one tile.

x is pre-deinterleaved on the host into even/odd column planes
[img, ch, 128, H, 2, 28], so the DVE transforms read unit-stride
operands directly from the DMA'd tiles (no on-chip deinterleave).
The head-critical DMAs are spread across the Sync/Act/DVE queues so
their descriptor writes run in parallel; all steady-state DMAs issue
from Sync (next image's x prefetched one image ahead) and the Act
queue carries only the PSUM drains.
"""

import json
import sys
import types

import numpy as np
import ml_dtypes

import concourse.bass as bass
import concourse.tile as tile
import concourse.mybir as mybir
from concourse.bass_utils import run_bass_kernel_spmd
from concourse.vector_clock import ScopedClock, VectorClock

if "antenv.axon_hooks" not in sys.modules:
    try:
        import antenv.axon_hooks  # noqa: F401
    except ImportError:
        _hooks = types.ModuleType("antenv.axon_hooks")

        def _get_hook(_cache=[]):
            if not _cache:
                try:
                    from trn_agent_boot.trn_boot import _ntff_profile_via_ctypes

                    _cache.append(_ntff_profile_via_ctypes("/opt/axon/libaxon_pjrt.so"))
                except Exception:
                    _cache.append(None)
            return _cache[0]

        _hooks.get_axon_ntff_profile_hook = _get_hook
        _hooks.set_axon_ntff_profile_hook = lambda h: None
        sys.modules["antenv.axon_hooks"] = _hooks
        try:
            import antenv

            antenv.axon_hooks = _hooks
        except ImportError:
            pass

N_CORES = 8
IMGS_PER_CORE = 4
C = 256
O = 256
H = W = 56
OH = OW = 54
KH = KW = 3
NTI = 27  # W-tiles (2 out cols each)
ROWS_PER_GROUP = 18
GCOLS = ROWS_PER_GROUP * NTI  # 486 matmul cols per plane per group
BF16 = mybir.dt.bfloat16
FP8 = mybir.dt.float8e4
F32 = mybir.dt.float32
ADD = mybir.AluOpType.add
SUB = mybir.AluOpType.subtract


class _SplitDrainTileContext(tile.TileContext):
    """The walrus build here rejects instructions carrying >2 semaphore
    waits; Tile's single kernel-tail drain accumulates one wait per
    outstanding logical proc. Split it into one drain per proc."""

    def _drain_and_barrier(self, tick_clock, wait_clock):
        g = tick_clock.global_clock
        n = len(g)
        for i in range(n):
            if g[i] == 0:
                continue
            vec = [0] * n
            vec[i] = g[i]
            d = self.nc.sync.drain()
            wait_clock.add_sem_waits(d.ins, ScopedClock({None: VectorClock(vec)}))

        self.nc.all_engine_barrier()
        assert self.sems is not None
        popped = self.nc._tile_sem_poison_stack.pop()
        assert popped is self._sem_poison
        self.nc.clear_and_free_semaphores(list(self.sems.allocated().values()))


def _split_sync_waits(bir_bytes):
    """Walrus allows only one semaphore wait on most instructions; hoist
    extras onto NoOps inserted before the instruction on the same engine."""
    m = json.loads(bir_bytes)
    ctr = 0
    for f in m["functions"]:
        for bb in f["blocks"]:
            out = []
            for inst in bb["instructions"]:
                si = inst.get("sync_info")
                waits = (si or {}).get("on_wait") or []
                if len(waits) > 1 and inst.get("opcode") != "EventSemaphore":
                    for w in waits[:-1]:
                        ctr += 1
                        nop = {
                            "engine": inst["engine"],
                            "ins": [],
                            "outs": [],
                            "name": f"SW-{ctr}",
                            "opcode": "NoOp",
                            "sync_info": {"on_update": [], "on_wait": [w]},
                        }
                        if "debug" in inst:
                            nop["debug"] = inst["debug"]
                        out.append(nop)
                    si["on_wait"] = [waits[-1]]
                out.append(inst)
            bb["instructions"] = out
    return json.dumps(m).encode()


N_WARMUP_MM = 16
XR_LO = 38  # input rows 0..37 serve out-row groups 0 (0..17) and 1 (18..35)
XR_HI = 20  # input rows 36..55 serve out-row group 2 (36..53)
HI_BASE = H - XR_HI  # 36


def build_program():
    nc = bass.Bass(
        trn_type="TRN2",
        target_bir_lowering=False,
        debug=False,
        enable_partition_id=False,
    )
    # x pre-deinterleaved on host: [img, ch, c128, row, parity, 28]
    x_d = nc.dram_tensor("x", [IMGS_PER_CORE, 2, 128, H, 2, 28], BF16, kind="ExternalInput")
    # transformed weights u: [c128, (ch, oh, kh, j, o128)] fp8 (exact values)
    u_d = nc.dram_tensor("u", [128, 2 * 2 * KH * 4 * 128], FP8, kind="ExternalInput")
    y_d = nc.dram_tensor(
        "y", [IMGS_PER_CORE, 2, 128, OH * OW], F32, kind="ExternalOutput"
    )

    with _SplitDrainTileContext(nc) as tc:
        with (
            tc.tile_pool(name="wpool", bufs=1) as wpool,
            tc.tile_pool(name="xpool", bufs=2) as xpool,
            tc.tile_pool(name="tpool", bufs=2) as tpool,
            tc.tile_pool(name="opool", bufs=1) as opool,
            tc.tile_pool(name="psumA", bufs=2, space="PSUM") as psA_pool,
            tc.tile_pool(name="psumB", bufs=2, space="PSUM") as psB_pool,
        ):
            ones_w = nc.const_aps.tensor(1.0, [128, 1], BF16)
            ones_r = nc.const_aps.tensor(1.0, [128, 128], BF16)
            ps_warm = psA_pool.tile([128, 2, 512], F32, name="ps_warm", tag="psA")
            for _ in range(N_WARMUP_MM):
                nc.tensor.matmul(
                    ps_warm[:1, 0, 0:128], ones_w, ones_r, start=True, stop=True
                )

            u_sb = wpool.tile([128, 2, 2, KH, 4, 128], FP8)
            u_r = u_d[:].rearrange(
                "p (ch oh kh j o) -> p ch oh kh j o", ch=2, oh=2, kh=KH, j=4
            )

            def alloc_img(img):
                x4 = {}
                tt = {}
                for ch in range(2):
                    for half, xr in ((0, XR_LO), (1, XR_HI)):
                        x4[ch, half] = xpool.tile(
                            [128, xr, 2, 28], BF16,
                            name=f"x{ch}{half}_{img}", tag=f"x{ch}{half}",
                        )
                        tt[ch, half] = tpool.tile(
                            [128, 4, xr, NTI], BF16,
                            name=f"t{ch}{half}_{img}", tag=f"t{ch}{half}",
                        )
                return x4, tt

            tiles = [alloc_img(0)]

            # img0-critical path: descriptor writes spread across queues so
            # they issue in parallel; rows 0..19 of both channels (enough
            # for out-row group 0) land first.
            x4_0 = tiles[0][0]
            nc.sync.dma_start(x4_0[0, 0][:, 0:11], x_d[0, 0, :, 0:11])
            nc.gpsimd.dma_start(x4_0[1, 0][:, 0:11], x_d[0, 1, :, 0:11])
            nc.sync.dma_start(u_sb[:, 0, 0], u_r[:, 0, 0])
            nc.scalar.dma_start(x4_0[0, 0][:, 11:20], x_d[0, 0, :, 11:20])
            nc.gpsimd.dma_start(x4_0[1, 0][:, 11:20], x_d[0, 1, :, 11:20])
            nc.sync.dma_start(u_sb[:, 1, 0], u_r[:, 1, 0])
            nc.scalar.dma_start(x4_0[0, 0][:, 20:XR_LO], x_d[0, 0, :, 20:XR_LO])
            nc.gpsimd.dma_start(x4_0[1, 0][:, 20:XR_LO], x_d[0, 1, :, 20:XR_LO])
            nc.sync.dma_start(u_sb[:, 0, 1], u_r[:, 0, 1])
            nc.sync.dma_start(u_sb[:, 1, 1], u_r[:, 1, 1])
            nc.sync.dma_start(x4_0[0, 1][:], x_d[0, 0, :, HI_BASE:H])
            nc.sync.dma_start(x4_0[1, 1][:], x_d[0, 1, :, HI_BASE:H])

            for img in range(IMGS_PER_CORE):
                x4, tt = tiles[img]

                def emit_tplane(ch, half, j, a, b):
                    # One Winograd t-plane row-chunk straight from the
                    # host-deinterleaved x tile (all operands unit-stride).
                    x_ = x4[ch, half]
                    t_ = tt[ch, half]
                    d0 = x_[:, a:b, 0, 0:NTI]
                    d1 = x_[:, a:b, 1, 0:NTI]
                    d2 = x_[:, a:b, 0, 1 : NTI + 1]
                    d3 = x_[:, a:b, 1, 1 : NTI + 1]
                    if j == 0:
                        nc.vector.tensor_tensor(t_[:, 0, a:b, :], d0, d2, SUB)
                    elif j == 1:
                        nc.vector.tensor_tensor(t_[:, 1, a:b, :], d1, d2, ADD)
                    elif j == 2:
                        nc.vector.tensor_tensor(t_[:, 2, a:b, :], d2, d1, SUB)
                    else:
                        nc.vector.tensor_tensor(t_[:, 3, a:b, :], d1, d3, SUB)

                def emit_transform(ch, half, a, b):
                    for j in range(4):
                        emit_tplane(ch, half, j, a, b)

                def run_group(img, oh_half, rg, tag_sfx, out_row0=None,
                              n_rows=ROWS_PER_GROUP, split=False):
                    tt_ = tiles[img][1]
                    if out_row0 is None:
                        out_row0 = rg * ROWS_PER_GROUP
                    gc = n_rows * NTI
                    hi = rg == 2
                    base_row = HI_BASE if hi else 0
                    # planes 1,2 in psA (freed early by the a/s drains),
                    # planes 0,3 in psB (freed by b_/ot1) -- two shallow
                    # pools so each 8KB group state double-buffers in PSUM.
                    psA = psA_pool.tile(
                        [128, 2, 512], F32, name=f"psA_{img}_{oh_half}_{tag_sfx}",
                        tag="psA",
                    )
                    psB = psB_pool.tile(
                        [128, 2, 512], F32, name=f"psB_{img}_{oh_half}_{tag_sfx}",
                        tag="psB",
                    )
                    pslot = {1: psA[:, 0], 2: psA[:, 1], 0: psB[:, 0], 3: psB[:, 1]}
                    r0 = out_row0 - base_row

                    def mm_plane(j):
                        k = 0
                        for ch in range(2):
                            t_ = tt_[ch, 1 if hi else 0]
                            for kh in range(KH):
                                nc.tensor.matmul(
                                    pslot[j][:, 0:gc],
                                    u_sb[:, ch, oh_half, kh, j, :],
                                    t_[:, j, r0 + kh : r0 + kh + n_rows, :],
                                    start=(k == 0),
                                    stop=(k == 2 * KH - 1),
                                )
                                k += 1

                    # Plane order j1, j2 first so the combine precursors
                    # (a=P1, s=P2, g=a-s) all run WHILE j0/j3 still stream:
                    # after the group's last matmul only `odd` (and the DMA)
                    # remain, so the PSUM tile recycles quickly.
                    mm_plane(1)
                    mm_plane(2)
                    ot = opool.tile(
                        [128, gc, 2], F32,
                        name=f"ot_{img}_{oh_half}_{tag_sfx}", tag="ot", bufs=5,
                    )
                    a_ = opool.tile(
                        [128, gc], F32, name=f"A_{img}_{oh_half}_{tag_sfx}",
                        tag="Asb", bufs=3,
                    )
                    b_ = opool.tile(
                        [128, gc], F32, name=f"B_{img}_{oh_half}_{tag_sfx}",
                        tag="Bsb", bufs=3,
                    )
                    s_ = opool.tile(
                        [128, gc], F32, name=f"S_{img}_{oh_half}_{tag_sfx}",
                        tag="Ssb", bufs=3,
                    )
                    g_ = opool.tile(
                        [128, gc], F32, name=f"G_{img}_{oh_half}_{tag_sfx}",
                        tag="Gsb", bufs=3,
                    )
                    # PSUM touches spread over engines: Act drains P1/P2,
                    # DVE does P0+a and g-P3, gpsimd the pure-SBUF ops.
                    nc.scalar.copy(a_[:], psA[:, 0, 0:gc])
                    nc.scalar.copy(s_[:], psA[:, 1, 0:gc])
                    nc.gpsimd.tensor_tensor(g_[:], a_[:], s_[:], SUB)
                    mm_plane(0)
                    mm_plane(3)
                    nc.vector.tensor_tensor(b_[:], psB[:, 0, 0:gc], a_[:], ADD)
                    nc.gpsimd.tensor_tensor(ot[:, :, 0], b_[:], s_[:], ADD)
                    nc.vector.tensor_tensor(
                        ot[:, :, 1], g_[:], psB[:, 1, 0:gc], SUB
                    )
                    e0 = out_row0 * OW
                    if split:
                        # kernel-ending group: contiguous halves on both
                        # queues so the two completion receipts overlap.
                        otf = ot[:].rearrange("p i two -> p (i two)")
                        nc.sync.dma_start(
                            y_d[img, oh_half, :, e0 : e0 + gc], otf[:, 0:gc]
                        )
                        nc.scalar.dma_start(
                            y_d[img, oh_half, :, e0 + gc : e0 + 2 * gc],
                            otf[:, gc : 2 * gc],
                        )
                    else:
                        nc.sync.dma_start(
                            y_d[img, oh_half, :, e0 : e0 + 2 * gc], ot[:]
                        )

                if img == 0:
                    # plane-priority order so group 0's first matmuls (plane
                    # 1 then 2) unblock after two DVE ops per channel.
                    for j in (1, 2, 0, 3):
                        emit_tplane(0, 0, j, 0, 11)
                        emit_tplane(1, 0, j, 0, 11)
                    for j in (1, 2, 0, 3):
                        emit_tplane(0, 0, j, 11, 20)
                        emit_tplane(1, 0, j, 11, 20)
                    emit_transform(0, 0, 20, XR_LO)
                    emit_transform(1, 0, 20, XR_LO)
                else:
                    emit_transform(0, 0, 0, XR_LO)
                    emit_transform(1, 0, 0, XR_LO)

                # Prefetch next image's x one image ahead so its DMA issues
                # precede this image's y-DMAs on the Sync queue.
                if img + 1 < IMGS_PER_CORE:
                    tiles.append(alloc_img(img + 1))
                    x4n = tiles[img + 1][0]
                    for ch in range(2):
                        nc.sync.dma_start(x4n[ch, 0][:], x_d[img + 1, ch, :, 0:XR_LO])
                    for ch in range(2):
                        nc.sync.dma_start(
                            x4n[ch, 1][:], x_d[img + 1, ch, :, HI_BASE:H]
                        )

                # lo groups for both o-halves, then hi. The hi-half
                # transforms are woven between groups in row chunks so they
                # never block the in-order helper-engine streams for long.
                hi_chunks = {0: (0, 1, 0, 10), 1: (1, 1, 0, 10),
                             2: (0, 1, 10, XR_HI), 3: (1, 1, 10, XR_HI)}
                for idx, (oh_half, rg) in enumerate(
                    [(o, r) for o in range(2) for r in range(2)]
                ):
                    if img == 0 and oh_half == 0 and rg == 0:
                        run_group(img, 0, 0, "0_0a", out_row0=0, n_rows=9)
                        run_group(img, 0, 0, "0_0b", out_row0=9, n_rows=9)
                    else:
                        run_group(img, oh_half, rg, f"{oh_half}_{rg}")
                    if idx in hi_chunks:
                        emit_transform(*hi_chunks[idx])
                last = img == IMGS_PER_CORE - 1
                for oh_half in range(2):
                    if last and oh_half == 1:
                        # kernel-ending group as 10+8 rows: the final
                        # combine+DMA chain handles only 216 cols/plane.
                        run_group(img, 1, 2, "1_2a", out_row0=36, n_rows=12)
                        run_group(img, 1, 2, "1_2b", out_row0=48, n_rows=6,
                                  split=True)
                    else:
                        run_group(img, oh_half, 2, f"{oh_half}_2")

    orig_to_json = nc.to_json_bytes
    nc.to_json_bytes = types.MethodType(
        lambda self: _split_sync_waits(orig_to_json()), nc
    )
    return nc


_NC = None


def _get_nc():
    global _NC
    if _NC is None:
        _NC = build_program()
    return _NC


def prepare_inputs(x, weights):
    """Full inputs -> list of 8 per-core input dicts (numpy)."""
    x = np.asarray(x, dtype=np.float32)
    weights = np.asarray(weights, dtype=np.float32)

    wb = np.where(weights >= 0, np.float32(1.0), np.float32(-1.0))
    G = np.array([[1, 0, 0], [0.5, 0.5, 0.5], [0.5, -0.5, 0.5], [0, 0, 1]],
                 np.float32)
    # u[o, c, kh, j] = sum_kw G[j, kw] * wb[o, c, kh, kw]
    u = np.einsum("jk,ochk->ochj", G, wb)
    # -> [c128, ch, oh, kh, j, o128]
    ut = u.reshape(2, 128, 2, 128, KH, 4)  # [oh, o128, ch, c128, kh, j]
    ut = ut.transpose(3, 2, 0, 4, 5, 1)  # [c128, ch, oh, kh, j, o128]
    u_core = np.ascontiguousarray(ut.reshape(128, -1)).astype(
        ml_dtypes.float8_e4m3fn
    )

    xr = x.reshape(N_CORES, IMGS_PER_CORE, 2, 128, H, 28, 2).astype(
        ml_dtypes.bfloat16
    )
    # host-side even/odd column deinterleave: [..., H, 28, 2] -> [..., H, 2, 28]
    xr = np.ascontiguousarray(xr.transpose(0, 1, 2, 3, 4, 6, 5))
    return [{"x": xr[i], "u": u_core} for i in range(N_CORES)]


def assemble(res):
    out = np.empty((32, O, OH, OW), dtype=np.float32)
    for i in range(N_CORES):
        out[i * IMGS_PER_CORE : (i + 1) * IMGS_PER_CORE] = res.results[i][
            "y"
        ].reshape(IMGS_PER_CORE, O, OH, OW)
    return out


def kernel(x, weights):
    nc = _get_nc()
    in_maps = prepare_inputs(x, weights)
    res = run_bass_kernel_spmd(nc, in_maps, core_ids=list(range(N_CORES)))
    return assemble(res)


# revision 15
# speedup vs baseline: 1.0264x; 1.0264x over previous
"""Binarized-weight 3x3 VALID conv via 1D Winograd F(2,3) along W, on 8
NeuronCores (data-parallel over batch, 4 images/core).

Per output row r and 2-col tile i (27 tiles = 54 cols):
  t0 = d[2i]-d[2i+2], t1 = d[2i+1]+d[2i+2], t2 = d[2i+2]-d[2i+1],
  t3 = d[2i+1]-d[2i+3]           (per channel, per input row; on DVE, bf16)
  P_j = sum_{c,kh} u_j[o,c,kh] * t_j[c, r+kh, i]   (PE matmuls, PSUM f32)
  out[2i] = P0+P1+P2,  out[2i+1] = P1-P2-P3        (DVE combine, f32)

u = G @ wb per kw-triple: values in {+-0.5, +-1, +-1.5} -- EXACT in
fp8-e4m3, so the stationary weights are fp8 (half-size LDWEIGHTS hides
behind the matmuls) while the moving t-planes stay bf16.

Output rows are processed in 18-row groups (486-col matmuls, one PSUM
bank per plane, 2 PSUM buffers) to halve per-instruction overhead on
every engine. x rows are split 0..37 / 36..55 so each group's 20-row
input window lives in one tile.

x is pre-deinterleaved on the host into even/odd column planes
[img, ch, 128, H, 2, 28], so the DVE transforms read unit-stride
operands directly from the DMA'd tiles (no on-chip deinterleave).
The head-critical DMAs are spread across the Sync/Act/DVE queues so
their descriptor writes run in parallel; all steady-state DMAs issue
from Sync (next image's x prefetched one image ahead) and the Act
queue carries only the PSUM drains.
"""

import json
import sys
import types

import numpy as np
import ml_dtypes

import concourse.bass as bass
import concourse.tile as tile
import concourse.mybir as mybir
from concourse.bass_utils import run_bass_kernel_spmd
from concourse.vector_clock import ScopedClock, VectorClock

if "antenv.axon_hooks" not in sys.modules:
    try:
        import antenv.axon_hooks  # noqa: F401
    except ImportError:
        _hooks = types.ModuleType("antenv.axon_hooks")

        def _get_hook(_cache=[]):
            if not _cache:
                try:
                    from trn_agent_boot.trn_boot import _ntff_profile_via_ctypes

                    _cache.append(_ntff_profile_via_ctypes("/opt/axon/libaxon_pjrt.so"))
                except Exception:
                    _cache.append(None)
            return _cache[0]

        _hooks.get_axon_ntff_profile_hook = _get_hook
        _hooks.set_axon_ntff_profile_hook = lambda h: None
        sys.modules["antenv.axon_hooks"] = _hooks
        try:
            import antenv

            antenv.axon_hooks = _hooks
        except ImportError:
            pass

N_CORES = 8
IMGS_PER_CORE = 4
C = 256
O = 256
H = W = 56
OH = OW = 54
KH = KW = 3
NTI = 27  # W-tiles (2 out cols each)
ROWS_PER_GROUP = 18
GCOLS = ROWS_PER_GROUP * NTI  # 486 matmul cols per plane per group
BF16 = mybir.dt.bfloat16
FP8 = mybir.dt.float8e4
F32 = mybir.dt.float32
ADD = mybir.AluOpType.add
SUB = mybir.AluOpType.subtract


class _SplitDrainTileContext(tile.TileContext):
    """The walrus build here rejects instructions carrying >2 semaphore
    waits; Tile's single kernel-tail drain accumulates one wait per
    outstanding logical proc. Split it into one drain per proc."""

    def _drain_and_barrier(self, tick_clock, wait_clock):
        g = tick_clock.global_clock
        n = len(g)
        for i in range(n):
            if g[i] == 0:
                continue
            vec = [0] * n
            vec[i] = g[i]
            d = self.nc.sync.drain()
            wait_clock.add_sem_waits(d.ins, ScopedClock({None: VectorClock(vec)}))

        self.nc.all_engine_barrier()
        assert self.sems is not None
        popped = self.nc._tile_sem_poison_stack.pop()
        assert popped is self._sem_poison
        self.nc.clear_and_free_semaphores(list(self.sems.allocated().values()))


def _split_sync_waits(bir_bytes):
    """Walrus allows only one semaphore wait on most instructions; hoist
    extras onto NoOps inserted before the instruction on the same engine."""
    m = json.loads(bir_bytes)
    ctr = 0
    for f in m["functions"]:
        for bb in f["blocks"]:
            out = []
            for inst in bb["instructions"]:
                si = inst.get("sync_info")
                waits = (si or {}).get("on_wait") or []
                if len(waits) > 1 and inst.get("opcode") != "EventSemaphore":
                    for w in waits[:-1]:
                        ctr += 1
                        nop = {
                            "engine": inst["engine"],
                            "ins": [],
                            "outs": [],
                            "name": f"SW-{ctr}",
                            "opcode": "NoOp",
                            "sync_info": {"on_update": [], "on_wait": [w]},
                        }
                        if "debug" in inst:
                            nop["debug"] = inst["debug"]
                        out.append(nop)
                    si["on_wait"] = [waits[-1]]
                out.append(inst)
            bb["instructions"] = out
    return json.dumps(m).encode()


N_WARMUP_MM = 20
XR_LO = 38  # input rows 0..37 serve out-row groups 0 (0..17) and 1 (18..35)
XR_HI = 20  # input rows 36..55 serve out-row group 2 (36..53)
HI_BASE = H - XR_HI  # 36


def build_program():
    nc = bass.Bass(
        trn_type="TRN2",
        target_bir_lowering=False,
        debug=False,
        enable_partition_id=False,
    )
    # x pre-deinterleaved on host: [img, ch, c128, row, parity, 28]
    x_d = nc.dram_tensor("x", [IMGS_PER_CORE, 2, 128, H, 2, 28], BF16, kind="ExternalInput")
    # transformed weights u: [c128, (ch, oh, kh, j, o128)] fp8 (exact values)
    u_d = nc.dram_tensor("u", [128, 2 * 2 * KH * 4 * 128], FP8, kind="ExternalInput")
    y_d = nc.dram_tensor(
        "y", [IMGS_PER_CORE, 2, 128, OH * OW], F32, kind="ExternalOutput"
    )

    with _SplitDrainTileContext(nc) as tc:
        with (
            tc.tile_pool(name="wpool", bufs=1) as wpool,
            tc.tile_pool(name="xpool", bufs=2) as xpool,
            tc.tile_pool(name="tpool", bufs=2) as tpool,
            tc.tile_pool(name="opool", bufs=1) as opool,
            tc.tile_pool(name="psumA", bufs=2, space="PSUM") as psA_pool,
            tc.tile_pool(name="psumB", bufs=2, space="PSUM") as psB_pool,
        ):
            ones_w = nc.const_aps.tensor(1.0, [128, 1], BF16)
            ones_r = nc.const_aps.tensor(1.0, [128, 128], BF16)
            ps_warm = psA_pool.tile([128, 2, 512], F32, name="ps_warm", tag="psA")
            for _ in range(N_WARMUP_MM):
                nc.tensor.matmul(
                    ps_warm[:1, 0, 0:128], ones_w, ones_r, start=True, stop=True
                )

            u_sb = wpool.tile([128, 2, 2, KH, 4, 128], FP8)
            u_r = u_d[:].rearrange(
                "p (ch oh kh j o) -> p ch oh kh j o", ch=2, oh=2, kh=KH, j=4
            )

            def alloc_img(img):
                x4 = {}
                tt = {}
                for ch in range(2):
                    for half, xr in ((0, XR_LO), (1, XR_HI)):
                        x4[ch, half] = xpool.tile(
                            [128, xr, 2, 28], BF16,
                            name=f"x{ch}{half}_{img}", tag=f"x{ch}{half}",
                        )
                        tt[ch, half] = tpool.tile(
                            [128, 4, xr, NTI], BF16,
                            name=f"t{ch}{half}_{img}", tag=f"t{ch}{half}",
                        )
                return x4, tt

            tiles = [alloc_img(0)]

            # img0-critical path: descriptor writes spread across queues so
            # they issue in parallel; rows 0..19 of both channels (enough
            # for out-row group 0) land first.
            x4_0 = tiles[0][0]
            nc.sync.dma_start(u_sb[:, 0, 0], u_r[:, 0, 0])
            nc.scalar.dma_start(x4_0[0, 0][:, 0:11], x_d[0, 0, :, 0:11])
            nc.gpsimd.dma_start(x4_0[1, 0][:, 0:11], x_d[0, 1, :, 0:11])
            nc.sync.dma_start(u_sb[:, 1, 0], u_r[:, 1, 0])
            nc.scalar.dma_start(x4_0[0, 0][:, 11:20], x_d[0, 0, :, 11:20])
            nc.gpsimd.dma_start(x4_0[1, 0][:, 11:20], x_d[0, 1, :, 11:20])
            nc.scalar.dma_start(x4_0[0, 0][:, 20:XR_LO], x_d[0, 0, :, 20:XR_LO])
            nc.gpsimd.dma_start(x4_0[1, 0][:, 20:XR_LO], x_d[0, 1, :, 20:XR_LO])
            nc.sync.dma_start(u_sb[:, 0, 1], u_r[:, 0, 1])
            nc.sync.dma_start(u_sb[:, 1, 1], u_r[:, 1, 1])
            nc.sync.dma_start(x4_0[0, 1][:], x_d[0, 0, :, HI_BASE:H])
            nc.sync.dma_start(x4_0[1, 1][:], x_d[0, 1, :, HI_BASE:H])

            for img in range(IMGS_PER_CORE):
                x4, tt = tiles[img]

                def emit_tplane(ch, half, j, a, b):
                    # One Winograd t-plane row-chunk straight from the
                    # host-deinterleaved x tile (all operands unit-stride).
                    x_ = x4[ch, half]
                    t_ = tt[ch, half]
                    d0 = x_[:, a:b, 0, 0:NTI]
                    d1 = x_[:, a:b, 1, 0:NTI]
                    d2 = x_[:, a:b, 0, 1 : NTI + 1]
                    d3 = x_[:, a:b, 1, 1 : NTI + 1]
                    if j == 0:
                        nc.vector.tensor_tensor(t_[:, 0, a:b, :], d0, d2, SUB)
                    elif j == 1:
                        nc.vector.tensor_tensor(t_[:, 1, a:b, :], d1, d2, ADD)
                    elif j == 2:
                        nc.vector.tensor_tensor(t_[:, 2, a:b, :], d2, d1, SUB)
                    else:
                        nc.vector.tensor_tensor(t_[:, 3, a:b, :], d1, d3, SUB)

                def emit_transform(ch, half, a, b):
                    for j in range(4):
                        emit_tplane(ch, half, j, a, b)

                def run_group(img, oh_half, rg, tag_sfx, out_row0=None,
                              n_rows=ROWS_PER_GROUP, split=False):
                    tt_ = tiles[img][1]
                    if out_row0 is None:
                        out_row0 = rg * ROWS_PER_GROUP
                    gc = n_rows * NTI
                    hi = rg == 2
                    base_row = HI_BASE if hi else 0
                    # planes 1,2 in psA (freed early by the a/s drains),
                    # planes 0,3 in psB (freed by b_/ot1) -- two shallow
                    # pools so each 8KB group state double-buffers in PSUM.
                    psA = psA_pool.tile(
                        [128, 2, 512], F32, name=f"psA_{img}_{oh_half}_{tag_sfx}",
                        tag="psA",
                    )
                    psB = psB_pool.tile(
                        [128, 2, 512], F32, name=f"psB_{img}_{oh_half}_{tag_sfx}",
                        tag="psB",
                    )
                    pslot = {1: psA[:, 0], 2: psA[:, 1], 0: psB[:, 0], 3: psB[:, 1]}
                    r0 = out_row0 - base_row

                    def mm_plane(j):
                        k = 0
                        for ch in range(2):
                            t_ = tt_[ch, 1 if hi else 0]
                            for kh in range(KH):
                                nc.tensor.matmul(
                                    pslot[j][:, 0:gc],
                                    u_sb[:, ch, oh_half, kh, j, :],
                                    t_[:, j, r0 + kh : r0 + kh + n_rows, :],
                                    start=(k == 0),
                                    stop=(k == 2 * KH - 1),
                                )
                                k += 1

                    # Plane order j1, j2 first so the combine precursors
                    # (a=P1, s=P2, g=a-s) all run WHILE j0/j3 still stream:
                    # after the group's last matmul only `odd` (and the DMA)
                    # remain, so the PSUM tile recycles quickly.
                    mm_plane(1)
                    mm_plane(2)
                    ot = opool.tile(
                        [128, gc, 2], F32,
                        name=f"ot_{img}_{oh_half}_{tag_sfx}", tag="ot", bufs=5,
                    )
                    a_ = opool.tile(
                        [128, gc], F32, name=f"A_{img}_{oh_half}_{tag_sfx}",
                        tag="Asb", bufs=3,
                    )
                    b_ = opool.tile(
                        [128, gc], F32, name=f"B_{img}_{oh_half}_{tag_sfx}",
                        tag="Bsb", bufs=3,
                    )
                    s_ = opool.tile(
                        [128, gc], F32, name=f"S_{img}_{oh_half}_{tag_sfx}",
                        tag="Ssb", bufs=3,
                    )
                    g_ = opool.tile(
                        [128, gc], F32, name=f"G_{img}_{oh_half}_{tag_sfx}",
                        tag="Gsb", bufs=3,
                    )
                    # PSUM touches spread over engines: Act drains P1/P2,
                    # DVE does P0+a and g-P3, gpsimd the pure-SBUF ops.
                    nc.scalar.copy(a_[:], psA[:, 0, 0:gc])
                    nc.scalar.copy(s_[:], psA[:, 1, 0:gc])
                    nc.gpsimd.tensor_tensor(g_[:], a_[:], s_[:], SUB)
                    mm_plane(0)
                    mm_plane(3)
                    nc.vector.tensor_tensor(b_[:], psB[:, 0, 0:gc], a_[:], ADD)
                    nc.gpsimd.tensor_tensor(ot[:, :, 0], b_[:], s_[:], ADD)
                    nc.vector.tensor_tensor(
                        ot[:, :, 1], g_[:], psB[:, 1, 0:gc], SUB
                    )
                    e0 = out_row0 * OW
                    if split:
                        # kernel-ending group: contiguous halves on both
                        # queues so the two completion receipts overlap.
                        otf = ot[:].rearrange("p i two -> p (i two)")
                        nc.sync.dma_start(
                            y_d[img, oh_half, :, e0 : e0 + gc], otf[:, 0:gc]
                        )
                        nc.scalar.dma_start(
                            y_d[img, oh_half, :, e0 + gc : e0 + 2 * gc],
                            otf[:, gc : 2 * gc],
                        )
                    else:
                        nc.sync.dma_start(
                            y_d[img, oh_half, :, e0 : e0 + 2 * gc], ot[:]
                        )

                if img == 0:
                    # plane-priority order so group 0's first matmuls (plane
                    # 1 then 2) unblock after two DVE ops per channel.
                    for j in (1, 2, 0, 3):
                        emit_tplane(0, 0, j, 0, 11)
                        emit_tplane(1, 0, j, 0, 11)
                    for j in (1, 2, 0, 3):
                        emit_tplane(0, 0, j, 11, 20)
                        emit_tplane(1, 0, j, 11, 20)
                    emit_transform(0, 0, 20, XR_LO)
                    emit_transform(1, 0, 20, XR_LO)
                else:
                    emit_transform(0, 0, 0, XR_LO)
                    emit_transform(1, 0, 0, XR_LO)

                # Prefetch next image's x one image ahead so its DMA issues
                # precede this image's y-DMAs on the Sync queue.
                if img + 1 < IMGS_PER_CORE:
                    tiles.append(alloc_img(img + 1))
                    x4n = tiles[img + 1][0]
                    for ch in range(2):
                        nc.sync.dma_start(x4n[ch, 0][:], x_d[img + 1, ch, :, 0:XR_LO])
                    for ch in range(2):
                        nc.sync.dma_start(
                            x4n[ch, 1][:], x_d[img + 1, ch, :, HI_BASE:H]
                        )

                # lo groups for both o-halves, then hi. The hi-half
                # transforms are woven between groups in row chunks so they
                # never block the in-order helper-engine streams for long.
                hi_chunks = {0: (0, 1, 0, 10), 1: (1, 1, 0, 10),
                             2: (0, 1, 10, XR_HI), 3: (1, 1, 10, XR_HI)}
                for idx, (oh_half, rg) in enumerate(
                    [(o, r) for o in range(2) for r in range(2)]
                ):
                    if img == 0 and oh_half == 0 and rg == 0:
                        run_group(img, 0, 0, "0_0a", out_row0=0, n_rows=9)
                        run_group(img, 0, 0, "0_0b", out_row0=9, n_rows=9)
                    else:
                        run_group(img, oh_half, rg, f"{oh_half}_{rg}")
                    if idx in hi_chunks:
                        emit_transform(*hi_chunks[idx])
                last = img == IMGS_PER_CORE - 1
                for oh_half in range(2):
                    if last and oh_half == 1:
                        # kernel-ending group as 10+8 rows: the final
                        # combine+DMA chain handles only 216 cols/plane.
                        run_group(img, 1, 2, "1_2a", out_row0=36, n_rows=12)
                        run_group(img, 1, 2, "1_2b", out_row0=48, n_rows=6,
                                  split=True)
                    else:
                        run_group(img, oh_half, 2, f"{oh_half}_2")

    orig_to_json = nc.to_json_bytes
    nc.to_json_bytes = types.MethodType(
        lambda self: _split_sync_waits(orig_to_json()), nc
    )
    return nc


_NC = None


def _get_nc():
    global _NC
    if _NC is None:
        _NC = build_program()
    return _NC


def prepare_inputs(x, weights):
    """Full inputs -> list of 8 per-core input dicts (numpy)."""
    x = np.asarray(x, dtype=np.float32)
    weights = np.asarray(weights, dtype=np.float32)

    wb = np.where(weights >= 0, np.float32(1.0), np.float32(-1.0))
    G = np.array([[1, 0, 0], [0.5, 0.5, 0.5], [0.5, -0.5, 0.5], [0, 0, 1]],
                 np.float32)
    # u[o, c, kh, j] = sum_kw G[j, kw] * wb[o, c, kh, kw]
    u = np.einsum("jk,ochk->ochj", G, wb)
    # -> [c128, ch, oh, kh, j, o128]
    ut = u.reshape(2, 128, 2, 128, KH, 4)  # [oh, o128, ch, c128, kh, j]
    ut = ut.transpose(3, 2, 0, 4, 5, 1)  # [c128, ch, oh, kh, j, o128]
    u_core = np.ascontiguousarray(ut.reshape(128, -1)).astype(
        ml_dtypes.float8_e4m3fn
    )

    xr = x.reshape(N_CORES, IMGS_PER_CORE, 2, 128, H, 28, 2).astype(
        ml_dtypes.bfloat16
    )
    # host-side even/odd column deinterleave: [..., H, 28, 2] -> [..., H, 2, 28]
    xr = np.ascontiguousarray(xr.transpose(0, 1, 2, 3, 4, 6, 5))
    return [{"x": xr[i], "u": u_core} for i in range(N_CORES)]


def assemble(res):
    out = np.empty((32, O, OH, OW), dtype=np.float32)
    for i in range(N_CORES):
        out[i * IMGS_PER_CORE : (i + 1) * IMGS_PER_CORE] = res.results[i][
            "y"
        ].reshape(IMGS_PER_CORE, O, OH, OW)
    return out


def kernel(x, weights):
    nc = _get_nc()
    in_maps = prepare_inputs(x, weights)
    res = run_bass_kernel_spmd(nc, in_maps, core_ids=list(range(N_CORES)))
    return assemble(res)


# revision 16
# speedup vs baseline: 1.0347x; 1.0080x over previous
"""Binarized-weight 3x3 VALID conv via 1D Winograd F(2,3) along W, on 8
NeuronCores (data-parallel over batch, 4 images/core).

Per output row r and 2-col tile i (27 tiles = 54 cols):
  t0 = d[2i]-d[2i+2], t1 = d[2i+1]+d[2i+2], t2 = d[2i+2]-d[2i+1],
  t3 = d[2i+1]-d[2i+3]           (per channel, per input row; on DVE, bf16)
  P_j = sum_{c,kh} u_j[o,c,kh] * t_j[c, r+kh, i]   (PE matmuls, PSUM f32)
  out[2i] = P0+P1+P2,  out[2i+1] = P1-P2-P3        (DVE combine, f32)

u = G @ wb per kw-triple: values in {+-0.5, +-1, +-1.5} -- EXACT in
fp8-e4m3, so the stationary weights are fp8 (half-size LDWEIGHTS hides
behind the matmuls) while the moving t-planes stay bf16.

Output rows are processed in 18-row groups (486-col matmuls, one PSUM
bank per plane, 2 PSUM buffers) to halve per-instruction overhead on
every engine. x rows are split 0..37 / 36..55 so each group's 20-row
input window lives in one tile.

x is pre-deinterleaved on the host into even/odd column planes
[img, ch, 128, H, 2, 28], so the DVE transforms read unit-stride
operands directly from the DMA'd tiles (no on-chip deinterleave).
The head-critical DMAs are spread across the Sync/Act/DVE queues so
their descriptor writes run in parallel; all steady-state DMAs issue
from Sync (next image's x prefetched one image ahead) and the Act
queue carries only the PSUM drains.
"""

import json
import sys
import types

import numpy as np
import ml_dtypes

import concourse.bass as bass
import concourse.tile as tile
import concourse.mybir as mybir
from concourse.bass_utils import run_bass_kernel_spmd
from concourse.vector_clock import ScopedClock, VectorClock

if "antenv.axon_hooks" not in sys.modules:
    try:
        import antenv.axon_hooks  # noqa: F401
    except ImportError:
        _hooks = types.ModuleType("antenv.axon_hooks")

        def _get_hook(_cache=[]):
            if not _cache:
                try:
                    from trn_agent_boot.trn_boot import _ntff_profile_via_ctypes

                    _cache.append(_ntff_profile_via_ctypes("/opt/axon/libaxon_pjrt.so"))
                except Exception:
                    _cache.append(None)
            return _cache[0]

        _hooks.get_axon_ntff_profile_hook = _get_hook
        _hooks.set_axon_ntff_profile_hook = lambda h: None
        sys.modules["antenv.axon_hooks"] = _hooks
        try:
            import antenv

            antenv.axon_hooks = _hooks
        except ImportError:
            pass

N_CORES = 8
IMGS_PER_CORE = 4
C = 256
O = 256
H = W = 56
OH = OW = 54
KH = KW = 3
NTI = 27  # W-tiles (2 out cols each)
ROWS_PER_GROUP = 18
GCOLS = ROWS_PER_GROUP * NTI  # 486 matmul cols per plane per group
BF16 = mybir.dt.bfloat16
FP8 = mybir.dt.float8e4
F32 = mybir.dt.float32
ADD = mybir.AluOpType.add
SUB = mybir.AluOpType.subtract


class _SplitDrainTileContext(tile.TileContext):
    """The walrus build here rejects instructions carrying >2 semaphore
    waits; Tile's single kernel-tail drain accumulates one wait per
    outstanding logical proc. Split it into one drain per proc."""

    def _drain_and_barrier(self, tick_clock, wait_clock):
        g = tick_clock.global_clock
        n = len(g)
        engs = [self.nc.sync, self.nc.scalar, self.nc.vector, self.nc.gpsimd,
                self.nc.tensor]
        k = 0
        for i in range(n):
            if g[i] == 0:
                continue
            vec = [0] * n
            vec[i] = g[i]
            d = engs[k % len(engs)].drain()
            k += 1
            wait_clock.add_sem_waits(d.ins, ScopedClock({None: VectorClock(vec)}))

        self.nc.all_engine_barrier()
        assert self.sems is not None
        popped = self.nc._tile_sem_poison_stack.pop()
        assert popped is self._sem_poison
        self.nc.clear_and_free_semaphores(list(self.sems.allocated().values()))


def _split_sync_waits(bir_bytes):
    """Walrus allows only one semaphore wait on most instructions; hoist
    extras onto NoOps inserted before the instruction on the same engine."""
    m = json.loads(bir_bytes)
    ctr = 0
    for f in m["functions"]:
        for bb in f["blocks"]:
            out = []
            for inst in bb["instructions"]:
                si = inst.get("sync_info")
                waits = (si or {}).get("on_wait") or []
                if len(waits) > 1 and inst.get("opcode") != "EventSemaphore":
                    for w in waits[:-1]:
                        ctr += 1
                        nop = {
                            "engine": inst["engine"],
                            "ins": [],
                            "outs": [],
                            "name": f"SW-{ctr}",
                            "opcode": "NoOp",
                            "sync_info": {"on_update": [], "on_wait": [w]},
                        }
                        if "debug" in inst:
                            nop["debug"] = inst["debug"]
                        out.append(nop)
                    si["on_wait"] = [waits[-1]]
                out.append(inst)
            bb["instructions"] = out
    return json.dumps(m).encode()


N_WARMUP_MM = 20
XR_LO = 38  # input rows 0..37 serve out-row groups 0 (0..17) and 1 (18..35)
XR_HI = 20  # input rows 36..55 serve out-row group 2 (36..53)
HI_BASE = H - XR_HI  # 36


def build_program():
    nc = bass.Bass(
        trn_type="TRN2",
        target_bir_lowering=False,
        debug=False,
        enable_partition_id=False,
    )
    # x pre-deinterleaved on host: [img, ch, c128, row, parity, 28]
    x_d = nc.dram_tensor("x", [IMGS_PER_CORE, 2, 128, H, 2, 28], BF16, kind="ExternalInput")
    # transformed weights u: [c128, (ch, oh, kh, j, o128)] fp8 (exact values)
    u_d = nc.dram_tensor("u", [128, 2 * 2 * KH * 4 * 128], FP8, kind="ExternalInput")
    y_d = nc.dram_tensor(
        "y", [IMGS_PER_CORE, 2, 128, OH * OW], F32, kind="ExternalOutput"
    )

    with _SplitDrainTileContext(nc) as tc:
        with (
            tc.tile_pool(name="wpool", bufs=1) as wpool,
            tc.tile_pool(name="xpool", bufs=2) as xpool,
            tc.tile_pool(name="tpool", bufs=2) as tpool,
            tc.tile_pool(name="opool", bufs=1) as opool,
            tc.tile_pool(name="psumA", bufs=2, space="PSUM") as psA_pool,
            tc.tile_pool(name="psumB", bufs=2, space="PSUM") as psB_pool,
        ):
            ones_w = nc.const_aps.tensor(1.0, [128, 1], BF16)
            ones_r = nc.const_aps.tensor(1.0, [128, 128], BF16)
            ps_warm = psA_pool.tile([128, 2, 512], F32, name="ps_warm", tag="psA")
            for _ in range(N_WARMUP_MM):
                nc.tensor.matmul(
                    ps_warm[:1, 0, 0:128], ones_w, ones_r, start=True, stop=True
                )

            u_sb = wpool.tile([128, 2, 2, KH, 4, 128], FP8)
            u_r = u_d[:].rearrange(
                "p (ch oh kh j o) -> p ch oh kh j o", ch=2, oh=2, kh=KH, j=4
            )

            def alloc_img(img):
                x4 = {}
                tt = {}
                for ch in range(2):
                    for half, xr in ((0, XR_LO), (1, XR_HI)):
                        x4[ch, half] = xpool.tile(
                            [128, xr, 2, 28], BF16,
                            name=f"x{ch}{half}_{img}", tag=f"x{ch}{half}",
                        )
                        tt[ch, half] = tpool.tile(
                            [128, 4, xr, NTI], BF16,
                            name=f"t{ch}{half}_{img}", tag=f"t{ch}{half}",
                        )
                return x4, tt

            tiles = [alloc_img(0)]

            # img0-critical path: descriptor writes spread across queues so
            # they issue in parallel; rows 0..19 of both channels (enough
            # for out-row group 0) land first.
            x4_0 = tiles[0][0]
            nc.sync.dma_start(u_sb[:, 0, 0], u_r[:, 0, 0])
            nc.scalar.dma_start(x4_0[0, 0][:, 0:11], x_d[0, 0, :, 0:11])
            nc.gpsimd.dma_start(x4_0[1, 0][:, 0:11], x_d[0, 1, :, 0:11])
            nc.sync.dma_start(u_sb[:, 1, 0], u_r[:, 1, 0])
            nc.scalar.dma_start(x4_0[0, 0][:, 11:20], x_d[0, 0, :, 11:20])
            nc.gpsimd.dma_start(x4_0[1, 0][:, 11:20], x_d[0, 1, :, 11:20])
            nc.scalar.dma_start(x4_0[0, 0][:, 20:XR_LO], x_d[0, 0, :, 20:XR_LO])
            nc.gpsimd.dma_start(x4_0[1, 0][:, 20:XR_LO], x_d[0, 1, :, 20:XR_LO])
            nc.sync.dma_start(u_sb[:, 0, 1], u_r[:, 0, 1])
            nc.sync.dma_start(u_sb[:, 1, 1], u_r[:, 1, 1])
            nc.sync.dma_start(x4_0[0, 1][:], x_d[0, 0, :, HI_BASE:H])
            nc.sync.dma_start(x4_0[1, 1][:], x_d[0, 1, :, HI_BASE:H])

            for img in range(IMGS_PER_CORE):
                x4, tt = tiles[img]

                def emit_tplane(ch, half, j, a, b):
                    # One Winograd t-plane row-chunk straight from the
                    # host-deinterleaved x tile (all operands unit-stride).
                    x_ = x4[ch, half]
                    t_ = tt[ch, half]
                    d0 = x_[:, a:b, 0, 0:NTI]
                    d1 = x_[:, a:b, 1, 0:NTI]
                    d2 = x_[:, a:b, 0, 1 : NTI + 1]
                    d3 = x_[:, a:b, 1, 1 : NTI + 1]
                    if j == 0:
                        nc.vector.tensor_tensor(t_[:, 0, a:b, :], d0, d2, SUB)
                    elif j == 1:
                        nc.vector.tensor_tensor(t_[:, 1, a:b, :], d1, d2, ADD)
                    elif j == 2:
                        nc.vector.tensor_tensor(t_[:, 2, a:b, :], d2, d1, SUB)
                    else:
                        nc.vector.tensor_tensor(t_[:, 3, a:b, :], d1, d3, SUB)

                def emit_transform(ch, half, a, b):
                    for j in range(4):
                        emit_tplane(ch, half, j, a, b)

                def run_group(img, oh_half, rg, tag_sfx, out_row0=None,
                              n_rows=ROWS_PER_GROUP, split=False):
                    tt_ = tiles[img][1]
                    if out_row0 is None:
                        out_row0 = rg * ROWS_PER_GROUP
                    gc = n_rows * NTI
                    hi = rg == 2
                    base_row = HI_BASE if hi else 0
                    # planes 1,2 in psA (freed early by the a/s drains),
                    # planes 0,3 in psB (freed by b_/ot1) -- two shallow
                    # pools so each 8KB group state double-buffers in PSUM.
                    psA = psA_pool.tile(
                        [128, 2, 512], F32, name=f"psA_{img}_{oh_half}_{tag_sfx}",
                        tag="psA",
                    )
                    psB = psB_pool.tile(
                        [128, 2, 512], F32, name=f"psB_{img}_{oh_half}_{tag_sfx}",
                        tag="psB",
                    )
                    pslot = {1: psA[:, 0], 2: psA[:, 1], 0: psB[:, 0], 3: psB[:, 1]}
                    r0 = out_row0 - base_row

                    def mm_plane(j):
                        k = 0
                        for ch in range(2):
                            t_ = tt_[ch, 1 if hi else 0]
                            for kh in range(KH):
                                nc.tensor.matmul(
                                    pslot[j][:, 0:gc],
                                    u_sb[:, ch, oh_half, kh, j, :],
                                    t_[:, j, r0 + kh : r0 + kh + n_rows, :],
                                    start=(k == 0),
                                    stop=(k == 2 * KH - 1),
                                )
                                k += 1

                    # Plane order j1, j2 first so the combine precursors
                    # (a=P1, s=P2, g=a-s) all run WHILE j0/j3 still stream:
                    # after the group's last matmul only `odd` (and the DMA)
                    # remain, so the PSUM tile recycles quickly.
                    mm_plane(1)
                    mm_plane(2)
                    ot = opool.tile(
                        [128, gc, 2], F32,
                        name=f"ot_{img}_{oh_half}_{tag_sfx}", tag="ot", bufs=5,
                    )
                    as_ = opool.tile(
                        [128, 2, gc], F32, name=f"AS_{img}_{oh_half}_{tag_sfx}",
                        tag="ASsb", bufs=3,
                    )
                    b_ = opool.tile(
                        [128, gc], F32, name=f"B_{img}_{oh_half}_{tag_sfx}",
                        tag="Bsb", bufs=3,
                    )
                    g_ = opool.tile(
                        [128, gc], F32, name=f"G_{img}_{oh_half}_{tag_sfx}",
                        tag="Gsb", bufs=3,
                    )
                    a_ = as_[:, 0]
                    s_ = as_[:, 1]
                    # PSUM touches spread over engines: Act drains P1+P2 in
                    # one pass, DVE does P0+a and g-P3, gpsimd the
                    # pure-SBUF ops.
                    nc.scalar.copy(as_[:], psA[:, 0:2, 0:gc])
                    nc.gpsimd.tensor_tensor(g_[:], a_[:], s_[:], SUB)
                    mm_plane(0)
                    mm_plane(3)
                    nc.vector.tensor_tensor(b_[:], psB[:, 0, 0:gc], a_[:], ADD)
                    nc.gpsimd.tensor_tensor(ot[:, :, 0], b_[:], s_[:], ADD)
                    nc.vector.tensor_tensor(
                        ot[:, :, 1], g_[:], psB[:, 1, 0:gc], SUB
                    )
                    e0 = out_row0 * OW
                    if split:
                        # kernel-ending group: contiguous halves on both
                        # queues so the two completion receipts overlap.
                        otf = ot[:].rearrange("p i two -> p (i two)")
                        nc.sync.dma_start(
                            y_d[img, oh_half, :, e0 : e0 + gc], otf[:, 0:gc]
                        )
                        nc.scalar.dma_start(
                            y_d[img, oh_half, :, e0 + gc : e0 + 2 * gc],
                            otf[:, gc : 2 * gc],
                        )
                    else:
                        nc.sync.dma_start(
                            y_d[img, oh_half, :, e0 : e0 + 2 * gc], ot[:]
                        )

                if img == 0:
                    # plane-priority order so group 0's first matmuls (plane
                    # 1 then 2) unblock after two DVE ops per channel.
                    for j in (1, 2, 0, 3):
                        emit_tplane(0, 0, j, 0, 11)
                        emit_tplane(1, 0, j, 0, 11)
                    for j in (1, 2, 0, 3):
                        emit_tplane(0, 0, j, 11, 20)
                        emit_tplane(1, 0, j, 11, 20)
                    emit_transform(0, 0, 20, XR_LO)
                    emit_transform(1, 0, 20, XR_LO)
                else:
                    emit_transform(0, 0, 0, XR_LO)
                    emit_transform(1, 0, 0, XR_LO)

                # Prefetch next image's x one image ahead so its DMA issues
                # precede this image's y-DMAs on the Sync queue.
                if img + 1 < IMGS_PER_CORE:
                    tiles.append(alloc_img(img + 1))
                    x4n = tiles[img + 1][0]
                    for ch in range(2):
                        nc.sync.dma_start(x4n[ch, 0][:], x_d[img + 1, ch, :, 0:XR_LO])
                    for ch in range(2):
                        nc.sync.dma_start(
                            x4n[ch, 1][:], x_d[img + 1, ch, :, HI_BASE:H]
                        )

                # lo groups for both o-halves, then hi. The hi-half
                # transforms are woven between groups in row chunks so they
                # never block the in-order helper-engine streams for long.
                hi_chunks = {0: (0, 1, 0, 10), 1: (1, 1, 0, 10),
                             2: (0, 1, 10, XR_HI), 3: (1, 1, 10, XR_HI)}
                for idx, (oh_half, rg) in enumerate(
                    [(o, r) for o in range(2) for r in range(2)]
                ):
                    if img == 0 and oh_half == 0 and rg == 0:
                        run_group(img, 0, 0, "0_0a", out_row0=0, n_rows=9)
                        run_group(img, 0, 0, "0_0b", out_row0=9, n_rows=9)
                    else:
                        run_group(img, oh_half, rg, f"{oh_half}_{rg}")
                    if idx in hi_chunks:
                        emit_transform(*hi_chunks[idx])
                last = img == IMGS_PER_CORE - 1
                for oh_half in range(2):
                    if last and oh_half == 1:
                        # kernel-ending group as 10+8 rows: the final
                        # combine+DMA chain handles only 216 cols/plane.
                        run_group(img, 1, 2, "1_2a", out_row0=36, n_rows=12)
                        run_group(img, 1, 2, "1_2b", out_row0=48, n_rows=6,
                                  split=True)
                    else:
                        run_group(img, oh_half, 2, f"{oh_half}_2")

    orig_to_json = nc.to_json_bytes
    nc.to_json_bytes = types.MethodType(
        lambda self: _split_sync_waits(orig_to_json()), nc
    )
    return nc


_NC = None


def _get_nc():
    global _NC
    if _NC is None:
        _NC = build_program()
    return _NC


def prepare_inputs(x, weights):
    """Full inputs -> list of 8 per-core input dicts (numpy)."""
    x = np.asarray(x, dtype=np.float32)
    weights = np.asarray(weights, dtype=np.float32)

    wb = np.where(weights >= 0, np.float32(1.0), np.float32(-1.0))
    G = np.array([[1, 0, 0], [0.5, 0.5, 0.5], [0.5, -0.5, 0.5], [0, 0, 1]],
                 np.float32)
    # u[o, c, kh, j] = sum_kw G[j, kw] * wb[o, c, kh, kw]
    u = np.einsum("jk,ochk->ochj", G, wb)
    # -> [c128, ch, oh, kh, j, o128]
    ut = u.reshape(2, 128, 2, 128, KH, 4)  # [oh, o128, ch, c128, kh, j]
    ut = ut.transpose(3, 2, 0, 4, 5, 1)  # [c128, ch, oh, kh, j, o128]
    u_core = np.ascontiguousarray(ut.reshape(128, -1)).astype(
        ml_dtypes.float8_e4m3fn
    )

    xr = x.reshape(N_CORES, IMGS_PER_CORE, 2, 128, H, 28, 2).astype(
        ml_dtypes.bfloat16
    )
    # host-side even/odd column deinterleave: [..., H, 28, 2] -> [..., H, 2, 28]
    xr = np.ascontiguousarray(xr.transpose(0, 1, 2, 3, 4, 6, 5))
    return [{"x": xr[i], "u": u_core} for i in range(N_CORES)]


def assemble(res):
    out = np.empty((32, O, OH, OW), dtype=np.float32)
    for i in range(N_CORES):
        out[i * IMGS_PER_CORE : (i + 1) * IMGS_PER_CORE] = res.results[i][
            "y"
        ].reshape(IMGS_PER_CORE, O, OH, OW)
    return out


def kernel(x, weights):
    nc = _get_nc()
    in_maps = prepare_inputs(x, weights)
    res = run_bass_kernel_spmd(nc, in_maps, core_ids=list(range(N_CORES)))
    return assemble(res)
